# revision 5
# baseline (speedup 1.0000x reference)
"""Bootstrapped BCE-with-logits loss (top-25% hard-pixel mining) on 8 TRN2 cores.

Math: loss_pixel = softplus(x) - x*y  (== max(x,0) - x*y + log1p(exp(-|x|)))
For each row, mean of top-k (k = N/4) pixel losses, then global mean.

Key identity: with t = k-th largest value of row v,
    sum_topk(row) = k*t + sum_j relu(v_j - t)
and the RHS is stationary in t at t* (d/dt = k - count(v > t) = 0), so an
approximate per-row threshold gives only O(rho*N*delta^2) error.  The kernel
estimates t per row with a Newton iteration on subsample counts (on-device),
then does one fused relu pass.

Sharding: data-parallel over the batch dim: core c handles rows 8c..8c+7.
Each core's 8 rows are laid out as SBUF [128 partitions x 16384], partition
p holding elements of row p//16.  Inputs are cast to bf16 on the host
(halves DMA traffic; validated rel-err ~5e-5 vs f32 reference).

Engine assignment (v2):
  ACT:   exp + ln(e+1)  (softplus; no native softplus LUT in this PWP build)
  DVE:   p = x*y (tt 2x), V = sp - p (tt 2x), rl = max(V - t, 0) (ts 4x),
         Newton count passes (ts 4x + accum)
  PE:    per-row accumulation of rl: psum[8, 512] += ind8.T @ rl_slice
         (replaces a 2nd DVE accum pass + shrinks the serial tail)
Per-core output is [8, 2] f32: col0 = per-row sum of relu(V - t),
col1 = per-row threshold t.  Host: total = sum(col0) + K*sum(col1);
answer = total / (B*K).
"""

import numpy as np
import ml_dtypes

_NCORES = 8
_B = 64
_HW = 512 * 512            # 262144 pixels per row
_RPC = _B // _NCORES       # 8 rows per core
_P = 128                   # SBUF partitions
_FREE = _RPC * _HW // _P   # 16384 elements per partition
# tapered chunk widths: small first chunk lets ACT start early, big middle
# chunks amortize per-op overhead, small last chunk shrinks the serial tail
_CHUNK_W = [1024, 2048, 2048, 2048, 4096, 4096, 1024]
_CHUNK_OFF = [sum(_CHUNK_W[:i]) for i in range(len(_CHUNK_W))]
_NCHUNK = len(_CHUNK_W)
assert sum(_CHUNK_W) == _FREE
_K = _HW // 4              # 65536 (top-k per row)
_PPR = _P // _RPC          # 16 partitions per row

# Newton refinement: (chunk index used for counting, sample width, half-width
# h, clamp on the update).  t0 is a distribution-informed initial guess.
_T0 = 0.92
_ROUNDS = [(1, 2048, 0.12, 0.50)]

_BF16 = ml_dtypes.bfloat16

# input-chunk double-buffer depth
_IO_BUFS = 3
# softplus-output buffer depth (ACT produces, DVE lags)
_SP_BUFS = 4
_ACC_COLS = 512            # PSUM accumulator lanes per row

_cached_nc = None


def build_bass(reps=1, loop=None):
    """Build the (SPMD, per-core identical) Bass program.

    reps > 1 repeats the whole body serially inside one NEFF; loop = N
    additionally wraps the body in a tc.For_i hardware loop (constant NEFF
    size at any rep count) — both only used for device-time measurement.
    """
    from concourse import bacc, mybir
    from concourse.tile import TileContext

    dt = mybir.dt
    Act = mybir.ActivationFunctionType
    Alu = mybir.AluOpType

    nc = bacc.Bacc("TRN2", target_bir_lowering=False, debug=False)

    # x and y interleaved per chunk ([..., j, 0, :] = x-chunk j, [..., 1, :] = y)
    # so each chunk needs exactly ONE dma_start.
    xy_ext = nc.declare_dram_parameter(
        "xy", [_P, 2 * _FREE], dt.bfloat16, isOutput=False
    )
    out_ext = nc.declare_dram_parameter("out", [_RPC, 2], dt.float32, isOutput=True)

    with TileContext(nc) as tc:
        with (
            tc.tile_pool(name="io", bufs=_IO_BUFS) as io_pool,
            tc.tile_pool(name="tmp", bufs=2) as tmp_pool,
            tc.tile_pool(name="spp", bufs=_SP_BUFS) as sp_pool,
            tc.tile_pool(name="persist", bufs=1) as persist,
            tc.tile_pool(name="small", bufs=1) as small,
            tc.tile_pool(name="psum", bufs=2, space="PSUM") as psum_pool,
            tc.tile_pool(name="psacc", bufs=1, space="PSUM") as psacc_pool,
        ):
            # persistent loss tile: all 8 rows of this core
            V = persist.tile([_P, _FREE], dt.bfloat16)

            # constants: row-indicator matrices for cross-partition
            # (per-row) reductions/broadcasts via the tensor engine.
            # ind8[p, b] = (p//16 == b), ind8T[b, p] = (p//16 == b)
            ind8 = small.tile([_P, _RPC], dt.float32)     # [128, 8] (Newton)
            ind8b = small.tile([_P, _RPC], dt.bfloat16)   # [128, 8] (finals)
            ind8T = small.tile([_RPC, _P], dt.float32)    # [8, 128]
            rid = small.tile([_P, 1], dt.int32)
            nc.gpsimd.iota(rid[:], [[0, 1]], channel_multiplier=1)
            nc.vector.tensor_scalar(
                rid[:], rid[:], 4, None, Alu.logical_shift_right
            )
            rid_f = small.tile([_P, 1], dt.float32)
            nc.vector.tensor_copy(rid_f[:], rid[:])
            col8 = small.tile([_P, _RPC], dt.int32)
            nc.gpsimd.iota(col8[:], [[1, _RPC]], channel_multiplier=0)
            col8_f = small.tile([_P, _RPC], dt.float32)
            nc.vector.tensor_copy(col8_f[:], col8[:])
            nc.vector.tensor_scalar(
                ind8[:], col8_f[:], rid_f[:], None, Alu.is_equal
            )
            nc.vector.tensor_copy(ind8b[:], ind8[:])
            colP = small.tile([_RPC, _P], dt.int32)
            nc.gpsimd.iota(colP[:], [[1, _P]], channel_multiplier=0)
            nc.vector.tensor_scalar(
                colP[:], colP[:], 4, None, Alu.logical_shift_right
            )
            rid8 = small.tile([_RPC, 1], dt.int32)
            nc.gpsimd.iota(rid8[:], [[0, 1]], channel_multiplier=1)
            rid8_f = small.tile([_RPC, 1], dt.float32)
            nc.vector.tensor_copy(rid8_f[:], rid8[:])
            colP_f = small.tile([_RPC, _P], dt.float32)
            nc.vector.tensor_copy(colP_f[:], colP[:])
            nc.vector.tensor_scalar(
                ind8T[:], colP_f[:], rid8_f[:], None, Alu.is_equal
            )

            # current per-row threshold, broadcast across the row's partitions
            t_bc = small.tile([_P, 1], dt.float32)
            t8 = small.tile([_RPC, 1], dt.float32)

            def produce_chunk(j):
                w, off = _CHUNK_W[j], _CHUNK_OFF[j]
                xyt = io_pool.tile([_P, 2 * w], dt.bfloat16, tag="xyt")
                nc.sync.dma_start(
                    xyt[:], xy_ext[:, 2 * off:2 * off + 2 * w]
                )
                xt = xyt[:, 0:w]
                yt = xyt[:, w:2 * w]
                sp = sp_pool.tile([_P, w], dt.bfloat16, tag="sp")
                e = tmp_pool.tile([_P, w], dt.float32, tag="e")
                nc.scalar.activation(e[:], xt, Act.Exp)
                nc.scalar.activation(sp[:], e[:], Act.Ln, bias=1.0)
                p = tmp_pool.tile([_P, w], dt.bfloat16, tag="p")
                nc.vector.tensor_tensor(p[:], xt, yt, Alu.mult)
                nc.vector.tensor_tensor(
                    V[:, off:off + w], sp[:], p[:], Alu.subtract
                )

            def newton_round(ridx, chunk, width, h, clamp):
                vc = V[:, _CHUNK_OFF[chunk]:_CHUNK_OFF[chunk] + width]
                n_samp = width * _PPR  # per-row sample count (over 16 parts)
                cnt = small.tile([_P, 3], dt.float32, tag=f"cnt{ridx}")
                msk = tmp_pool.tile([_P, width], dt.bfloat16, tag=f"msk{ridx}")
                # count = sum is_ge(v, thr); 3 thresholds around t0
                assert ridx == 0
                for i, off in enumerate((-h, 0.0, h)):
                    nc.vector.tensor_scalar(
                        msk[:], vc, float(_T0 + off), None, Alu.is_ge,
                        Alu.add, accum_out=cnt[:, i:i + 1],
                    )
                # per-row counts: [8, 3] = ind8.T @ cnt
                pc = psum_pool.tile([_RPC, 3], dt.float32, tag="pc")
                nc.tensor.matmul(pc[:], ind8[:], cnt[:])
                rc = small.tile([_RPC, 3], dt.float32, tag=f"rc{ridx}")
                nc.vector.tensor_copy(rc[:], pc[:])
                # Newton update: t += clamp(2h*(c_mid - n/4)/(c_lo - c_hi))
                num = small.tile([_RPC, 1], dt.float32, tag=f"num{ridx}")
                den = small.tile([_RPC, 1], dt.float32, tag=f"den{ridx}")
                q = small.tile([_RPC, 1], dt.float32, tag=f"q{ridx}")
                nc.vector.tensor_scalar(
                    num[:], rc[:, 1:2], float(n_samp / 4), float(2.0 * h),
                    Alu.subtract, Alu.mult,
                )
                nc.vector.tensor_tensor(den[:], rc[:, 0:1], rc[:, 2:3], Alu.subtract)
                rden = small.tile([_RPC, 1], dt.float32, tag=f"rden{ridx}")
                nc.vector.reciprocal(rden[:], den[:])
                nc.vector.tensor_tensor(q[:], num[:], rden[:], Alu.mult)
                nc.vector.tensor_scalar(
                    q[:], q[:], float(clamp), float(-clamp), Alu.min, Alu.max
                )
                nc.vector.tensor_tensor(t8[:], t8[:], q[:], Alu.add)
                # broadcast t8 [8,1] -> t_bc [128,1]
                pt = psum_pool.tile([_P, 1], dt.float32, tag="pt")
                nc.tensor.matmul(pt[:], ind8T[:], t8[:])
                nc.vector.tensor_copy(t_bc[:], pt[:])

            # final pass: rl = max(V - t, 0) on DVE (ts 4x), then per-row
            # accumulation on the tensor engine: ps[8, 512] += ind8b.T @ rl
            n_mm_total = sum(
                (w + _ACC_COLS - 1) // _ACC_COLS for w in _CHUNK_W
            )

            def make_final(ps, mm_state):
                def final_chunk(j):
                    w, off = _CHUNK_W[j], _CHUNK_OFF[j]
                    rl = tmp_pool.tile([_P, w], dt.bfloat16, tag="rl")
                    nc.vector.tensor_scalar(
                        rl[:], V[:, off:off + w], t_bc[:], 0.0,
                        Alu.subtract, Alu.max,
                    )
                    for s in range(0, w, _ACC_COLS):
                        sw = min(_ACC_COLS, w - s)
                        nc.tensor.matmul(
                            ps[:, 0:sw], ind8b[:], rl[:, s:s + sw],
                            start=(mm_state[0] == 0),
                            stop=(mm_state[0] == n_mm_total - 1),
                        )
                        mm_state[0] += 1
                return final_chunk

            round_chunks = max(r[0] for r in _ROUNDS) + 1 if _ROUNDS else 0

            def rep_body():
                nc.vector.memset(t_bc[:], _T0)
                nc.vector.memset(t8[:], _T0)
                ps = psacc_pool.tile([_RPC, _ACC_COLS], dt.float32, tag="ps")
                mm_state = [0]
                final_chunk = make_final(ps, mm_state)
                for j in range(round_chunks):
                    produce_chunk(j)
                for ridx, (chunk, width, h, clamp) in enumerate(_ROUNDS):
                    newton_round(ridx, chunk, width, h, clamp)
                for j in range(round_chunks):
                    final_chunk(j)
                for j in range(round_chunks, _NCHUNK):
                    produce_chunk(j)
                    final_chunk(j)
                assert mm_state[0] == n_mm_total

                rs = small.tile([_RPC, _ACC_COLS], dt.float32, tag="rs")
                nc.vector.tensor_copy(rs[:], ps[:])
                out_t = small.tile([_RPC, 2], dt.float32, tag="out_t")
                nc.vector.tensor_reduce(
                    out_t[:, 0:1], rs[:], mybir.AxisListType.X, Alu.add
                )
                nc.vector.tensor_copy(out_t[:, 1:2], t8[:])
                nc.sync.dma_start(out_ext[:], out_t[:])

            if loop is None:
                for _rep in range(reps):
                    rep_body()
            else:
                with tc.For_i(0, loop, 1):
                    for _rep in range(reps):
                        rep_body()

    # Steer the ACT table chooser to the one set holding BOTH Exp and Ln so
    # it loads a single table instead of thrashing exp<->ln sets per chunk.
    from concourse import bacc as _bacc_mod
    _orig_tables = _bacc_mod.get_activation_tables

    def _steered_tables(arch):
        tabs = dict(_orig_tables(arch))
        used = {Act.Exp, Act.Ln, Act.Sign, Act.Relu}
        combined = [n for n, fns in tabs.items() if {Act.Exp, Act.Ln} <= fns]
        if combined:
            keep = combined[0]
            tabs = {
                n: (fns if n == keep else (fns - used))
                for n, fns in tabs.items()
            }
        return tabs

    _bacc_mod.get_activation_tables = _steered_tables
    try:
        nc.compile()
    finally:
        _bacc_mod.get_activation_tables = _orig_tables
    return nc


def _shard_inputs(pred_logits, gts):
    x = np.ascontiguousarray(pred_logits, dtype=np.float32).reshape(_B, _HW)
    y = np.ascontiguousarray(gts, dtype=np.float32).reshape(_B, _HW)
    xb = x.astype(_BF16)
    yb = y.astype(_BF16)
    in_maps = []
    for c in range(_NCORES):
        sl = slice(c * _RPC, (c + 1) * _RPC)
        xs = xb[sl].reshape(_P, _FREE)
        ys = yb[sl].reshape(_P, _FREE)
        # interleave x/y per (variable-width) chunk: [... x_w | y_w ...]
        blocks = []
        for w, off in zip(_CHUNK_W, _CHUNK_OFF):
            blocks.append(xs[:, off:off + w])
            blocks.append(ys[:, off:off + w])
        xy = np.concatenate(blocks, axis=1)
        in_maps.append({"xy": np.ascontiguousarray(xy)})
    return in_maps


def _combine(results):
    total = 0.0
    for c in range(_NCORES):
        out = np.asarray(results[c]["out"], dtype=np.float64)  # [8, 2]
        total += out[:, 0].sum()
        total += _K * out[:, 1].sum()
    return np.float32(total / (_B * _K))


def kernel(pred_logits, gts):
    from concourse.bass_utils import run_bass_kernel_spmd

    global _cached_nc
    if _cached_nc is None:
        _cached_nc = build_bass()
    in_maps = _shard_inputs(pred_logits, gts)
    res = run_bass_kernel_spmd(_cached_nc, in_maps, list(range(_NCORES)))
    return _combine(res.results)


# revision 11
# speedup vs baseline: 2.8328x; 2.8328x over previous
"""Bootstrapped BCE-with-logits loss (top-25% hard-pixel mining) on 8 TRN2 cores.

Math: loss_pixel = softplus(x) - x*y  (== max(x,0) - x*y + log1p(exp(-|x|)))
For each row, mean of top-k (k = N/4) pixel losses, then global mean.

Key identity: with t = k-th largest value of row v,
    sum_topk(row) = k*t + sum_j relu(v_j - t)
and the RHS is stationary in t at t* (d/dt = k - count(v > t) = 0), so an
approximate per-row threshold gives only O(rho*N*delta^2) error.  The kernel
estimates t per row with a Newton iteration on subsample counts (on-device),
then does one fused relu pass.

Sharding: data-parallel over the batch dim: core c handles rows 8c..8c+7.
Each core's 8 rows are laid out as SBUF [128 partitions x 16384], partition
p holding elements of row p//16.  Inputs are cast to bf16 on the host
(halves DMA traffic; validated rel-err ~5e-5 vs f32 reference).

Engine assignment (v2):
  ACT:   exp + ln(e+1)  (softplus; no native softplus LUT in this PWP build)
  DVE:   p = x*y (tt 2x), V = sp - p (tt 2x), rl = max(V - t, 0) (ts 4x),
         Newton count passes (ts 4x + accum)
  PE:    per-row accumulation of rl: psum[8, 512] += ind8.T @ rl_slice
         (replaces a 2nd DVE accum pass + shrinks the serial tail)
Per-core output is [8, 2] f32: col0 = per-row sum of relu(V - t),
col1 = per-row threshold t.  Host: total = sum(col0) + K*sum(col1);
answer = total / (B*K).
"""

import numpy as np
import ml_dtypes

_NCORES = 8
_B = 64
_HW = 512 * 512            # 262144 pixels per row
_RPC = _B // _NCORES       # 8 rows per core
_P = 128                   # SBUF partitions
_FREE = _RPC * _HW // _P   # 16384 elements per partition
# tapered chunk widths: small first chunk lets ACT start early, equal middle
# chunks keep DMA ahead of ACT, small last chunk shrinks the serial tail
_CHUNK_W = [1024, 2048, 2048, 2048, 2048, 2048, 2048, 2048, 1024]
_CHUNK_OFF = [sum(_CHUNK_W[:i]) for i in range(len(_CHUNK_W))]
_NCHUNK = len(_CHUNK_W)
assert sum(_CHUNK_W) == _FREE
_K = _HW // 4              # 65536 (top-k per row)
_PPR = _P // _RPC          # 16 partitions per row

# chunk indices whose final (relu+accumulate) runs on the ACT engine
# (per-partition accum, summed on host) instead of DVE+PE: the trailing
# chunks, so the serial tail after the last Ln is one short ACT pass.
_ACT_FIN = frozenset({_NCHUNK - 1})

# Newton refinement: (chunk index used for counting, sample width, half-width
# h, clamp on the update).  t0 is a distribution-informed initial guess.
_T0 = 0.92
_ROUNDS = [(1, 2048, 0.12, 0.50)]

_BF16 = ml_dtypes.bfloat16

# input-chunk double-buffer depth
_IO_BUFS = 3
# softplus-output buffer depth (ACT produces, DVE lags)
_SP_BUFS = 4
_ACC_COLS = 512            # PSUM accumulator lanes per row

_cached_nc = None


def build_bass(reps=1, loop=None):
    """Build the (SPMD, per-core identical) Bass program.

    reps > 1 repeats the whole body serially inside one NEFF; loop = N
    additionally wraps the body in a tc.For_i hardware loop (constant NEFF
    size at any rep count) — both only used for device-time measurement.
    """
    from concourse import bacc, mybir
    from concourse.tile import TileContext

    dt = mybir.dt
    Act = mybir.ActivationFunctionType
    Alu = mybir.AluOpType

    nc = bacc.Bacc("TRN2", target_bir_lowering=False, debug=False)

    # x and y interleaved per chunk ([..., j, 0, :] = x-chunk j, [..., 1, :] = y)
    # so each chunk needs exactly ONE dma_start.
    xy_ext = nc.declare_dram_parameter(
        "xy", [_P, 2 * _FREE], dt.bfloat16, isOutput=False
    )
    out_ext = nc.declare_dram_parameter("out", [_RPC, 2], dt.float32, isOutput=True)
    # per-partition relu sums from the ACT-final chunks (host sums them)
    n_act_fin = len(_ACT_FIN)
    out2_ext = nc.declare_dram_parameter(
        "out2", [_P, max(n_act_fin, 1)], dt.float32, isOutput=True
    )

    with TileContext(nc) as tc:
        with (
            tc.tile_pool(name="io", bufs=_IO_BUFS) as io_pool,
            tc.tile_pool(name="tmp", bufs=2) as tmp_pool,
            tc.tile_pool(name="spp", bufs=_SP_BUFS) as sp_pool,
            tc.tile_pool(name="persist", bufs=1) as persist,
            tc.tile_pool(name="small", bufs=1) as small,
            tc.tile_pool(name="psum", bufs=2, space="PSUM") as psum_pool,
            tc.tile_pool(name="psacc", bufs=1, space="PSUM") as psacc_pool,
        ):
            # persistent loss tile: all 8 rows of this core
            V = persist.tile([_P, _FREE], dt.bfloat16)

            # constants: row-indicator matrices for cross-partition
            # (per-row) reductions/broadcasts via the tensor engine.
            # ind8[p, b] = (p//16 == b), ind8T[b, p] = (p//16 == b)
            ind8 = small.tile([_P, _RPC], dt.float32)     # [128, 8] (Newton)
            ind8b = small.tile([_P, _RPC], dt.bfloat16)   # [128, 8] (finals)
            ind8T = small.tile([_RPC, _P], dt.float32)    # [8, 128]
            rid = small.tile([_P, 1], dt.int32)
            nc.gpsimd.iota(rid[:], [[0, 1]], channel_multiplier=1)
            nc.vector.tensor_scalar(
                rid[:], rid[:], 4, None, Alu.logical_shift_right
            )
            rid_f = small.tile([_P, 1], dt.float32)
            nc.vector.tensor_copy(rid_f[:], rid[:])
            col8 = small.tile([_P, _RPC], dt.int32)
            nc.gpsimd.iota(col8[:], [[1, _RPC]], channel_multiplier=0)
            col8_f = small.tile([_P, _RPC], dt.float32)
            nc.vector.tensor_copy(col8_f[:], col8[:])
            nc.vector.tensor_scalar(
                ind8[:], col8_f[:], rid_f[:], None, Alu.is_equal
            )
            nc.vector.tensor_copy(ind8b[:], ind8[:])
            colP = small.tile([_RPC, _P], dt.int32)
            nc.gpsimd.iota(colP[:], [[1, _P]], channel_multiplier=0)
            nc.vector.tensor_scalar(
                colP[:], colP[:], 4, None, Alu.logical_shift_right
            )
            rid8 = small.tile([_RPC, 1], dt.int32)
            nc.gpsimd.iota(rid8[:], [[0, 1]], channel_multiplier=1)
            rid8_f = small.tile([_RPC, 1], dt.float32)
            nc.vector.tensor_copy(rid8_f[:], rid8[:])
            colP_f = small.tile([_RPC, _P], dt.float32)
            nc.vector.tensor_copy(colP_f[:], colP[:])
            nc.vector.tensor_scalar(
                ind8T[:], colP_f[:], rid8_f[:], None, Alu.is_equal
            )

            # current per-row threshold, broadcast across the row's partitions
            t_bc = small.tile([_P, 1], dt.float32)
            t8 = small.tile([_RPC, 1], dt.float32)
            neg_t = small.tile([_P, 1], dt.float32)

            def produce_chunk(j):
                w, off = _CHUNK_W[j], _CHUNK_OFF[j]
                xyt = io_pool.tile([_P, 2 * w], dt.bfloat16, tag="xyt")
                nc.sync.dma_start(
                    xyt[:], xy_ext[:, 2 * off:2 * off + 2 * w]
                )
                xt = xyt[:, 0:w]
                yt = xyt[:, w:2 * w]
                sp = sp_pool.tile([_P, w], dt.bfloat16, tag="sp")
                e = tmp_pool.tile([_P, w], dt.float32, tag="e")
                nc.scalar.activation(e[:], xt, Act.Exp)
                nc.scalar.activation(sp[:], e[:], Act.Ln, bias=1.0)
                p = tmp_pool.tile([_P, w], dt.bfloat16, tag="p")
                nc.vector.tensor_tensor(p[:], xt, yt, Alu.mult)
                nc.vector.tensor_tensor(
                    V[:, off:off + w], sp[:], p[:], Alu.subtract
                )

            def newton_round(ridx, chunk, width, h, clamp):
                vc = V[:, _CHUNK_OFF[chunk]:_CHUNK_OFF[chunk] + width]
                n_samp = width * _PPR  # per-row sample count (over 16 parts)
                cnt = small.tile([_P, 3], dt.float32, tag=f"cnt{ridx}")
                msk = tmp_pool.tile([_P, width], dt.bfloat16, tag=f"msk{ridx}")
                # count = sum is_ge(v, thr); 3 thresholds around t0
                assert ridx == 0
                for i, off in enumerate((-h, 0.0, h)):
                    nc.vector.tensor_scalar(
                        msk[:], vc, float(_T0 + off), None, Alu.is_ge,
                        Alu.add, accum_out=cnt[:, i:i + 1],
                    )
                # per-row counts: [8, 3] = ind8.T @ cnt
                pc = psum_pool.tile([_RPC, 3], dt.float32, tag="pc")
                nc.tensor.matmul(pc[:], ind8[:], cnt[:])
                rc = small.tile([_RPC, 3], dt.float32, tag=f"rc{ridx}")
                nc.vector.tensor_copy(rc[:], pc[:])
                # Newton update: t += clamp(2h*(c_mid - n/4)/(c_lo - c_hi))
                num = small.tile([_RPC, 1], dt.float32, tag=f"num{ridx}")
                den = small.tile([_RPC, 1], dt.float32, tag=f"den{ridx}")
                q = small.tile([_RPC, 1], dt.float32, tag=f"q{ridx}")
                nc.vector.tensor_scalar(
                    num[:], rc[:, 1:2], float(n_samp / 4), float(2.0 * h),
                    Alu.subtract, Alu.mult,
                )
                nc.vector.tensor_tensor(den[:], rc[:, 0:1], rc[:, 2:3], Alu.subtract)
                rden = small.tile([_RPC, 1], dt.float32, tag=f"rden{ridx}")
                nc.vector.reciprocal(rden[:], den[:])
                nc.vector.tensor_tensor(q[:], num[:], rden[:], Alu.mult)
                nc.vector.tensor_scalar(
                    q[:], q[:], float(clamp), float(-clamp), Alu.min, Alu.max
                )
                nc.vector.tensor_tensor(t8[:], t8[:], q[:], Alu.add)
                # broadcast t8 [8,1] -> t_bc [128,1]
                pt = psum_pool.tile([_P, 1], dt.float32, tag="pt")
                nc.tensor.matmul(pt[:], ind8T[:], t8[:])
                nc.vector.tensor_copy(t_bc[:], pt[:])

            # final pass: rl = max(V - t, 0) on DVE (ts 4x), then per-row
            # accumulation on the tensor engine: ps[8, 512] += ind8b.T @ rl.
            # Chunks in _ACT_FIN instead run Relu(bias=-t)+accum on ACT
            # (per-partition sums, combined on host) — shrinks the tail.
            n_mm_total = sum(
                (w + _ACC_COLS - 1) // _ACC_COLS
                for j, w in enumerate(_CHUNK_W) if j not in _ACT_FIN
            )

            def make_final(ps, acc, mm_state):
                def final_chunk(j):
                    w, off = _CHUNK_W[j], _CHUNK_OFF[j]
                    if j in _ACT_FIN:
                        ci = sorted(_ACT_FIN).index(j)
                        rl = tmp_pool.tile([_P, w], dt.bfloat16, tag="rl")
                        nc.scalar.activation(
                            rl[:], V[:, off:off + w], Act.Relu,
                            bias=neg_t[:], accum_out=acc[:, ci:ci + 1],
                        )
                        return
                    rl = tmp_pool.tile([_P, w], dt.bfloat16, tag="rl")
                    nc.vector.tensor_scalar(
                        rl[:], V[:, off:off + w], t_bc[:], 0.0,
                        Alu.subtract, Alu.max,
                    )
                    for s in range(0, w, _ACC_COLS):
                        sw = min(_ACC_COLS, w - s)
                        nc.tensor.matmul(
                            ps[:, 0:sw], ind8b[:], rl[:, s:s + sw],
                            start=(mm_state[0] == 0),
                            stop=(mm_state[0] == n_mm_total - 1),
                        )
                        mm_state[0] += 1
                return final_chunk

            round_chunks = max(r[0] for r in _ROUNDS) + 1 if _ROUNDS else 0

            def rep_body():
                nc.vector.memset(t_bc[:], _T0)
                nc.vector.memset(t8[:], _T0)
                ps = psacc_pool.tile([_RPC, _ACC_COLS], dt.float32, tag="ps")
                acc = small.tile([_P, max(n_act_fin, 1)], dt.float32, tag="acc")
                if n_act_fin == 0:
                    nc.vector.memset(acc[:], 0.0)
                mm_state = [0]
                final_chunk = make_final(ps, acc, mm_state)
                for j in range(round_chunks):
                    produce_chunk(j)
                for ridx, (chunk, width, h, clamp) in enumerate(_ROUNDS):
                    newton_round(ridx, chunk, width, h, clamp)
                nc.vector.tensor_scalar_mul(neg_t[:], t_bc[:], -1.0)
                for j in range(round_chunks):
                    final_chunk(j)
                for j in range(round_chunks, _NCHUNK):
                    produce_chunk(j)
                    final_chunk(j)
                assert mm_state[0] == n_mm_total

                out_t = small.tile([_RPC, 2], dt.float32, tag="out_t")
                nc.vector.tensor_reduce(
                    out_t[:, 0:1], ps[:], mybir.AxisListType.X, Alu.add
                )
                nc.vector.tensor_copy(out_t[:, 1:2], t8[:])
                nc.sync.dma_start(out_ext[:], out_t[:])
                nc.sync.dma_start(out2_ext[:], acc[:])

            if loop is None:
                for _rep in range(reps):
                    rep_body()
            else:
                with tc.For_i(0, loop, 1):
                    for _rep in range(reps):
                        rep_body()

    # Steer the ACT table chooser to the one set holding BOTH Exp and Ln so
    # it loads a single table instead of thrashing exp<->ln sets per chunk.
    from concourse import bacc as _bacc_mod
    _orig_tables = _bacc_mod.get_activation_tables

    def _steered_tables(arch):
        tabs = dict(_orig_tables(arch))
        used = {Act.Exp, Act.Ln, Act.Sign, Act.Relu}
        combined = [n for n, fns in tabs.items() if {Act.Exp, Act.Ln} <= fns]
        if combined:
            keep = combined[0]
            tabs = {
                n: (fns if n == keep else (fns - used))
                for n, fns in tabs.items()
            }
        return tabs

    _bacc_mod.get_activation_tables = _steered_tables
    try:
        nc.compile()
    finally:
        _bacc_mod.get_activation_tables = _orig_tables
    return nc


def _shard_inputs(pred_logits, gts):
    x = np.ascontiguousarray(pred_logits, dtype=np.float32).reshape(_B, _HW)
    y = np.ascontiguousarray(gts, dtype=np.float32).reshape(_B, _HW)
    xb = x.astype(_BF16)
    yb = y.astype(_BF16)
    in_maps = []
    for c in range(_NCORES):
        sl = slice(c * _RPC, (c + 1) * _RPC)
        xs = xb[sl].reshape(_P, _FREE)
        ys = yb[sl].reshape(_P, _FREE)
        # interleave x/y per (variable-width) chunk: [... x_w | y_w ...]
        blocks = []
        for w, off in zip(_CHUNK_W, _CHUNK_OFF):
            blocks.append(xs[:, off:off + w])
            blocks.append(ys[:, off:off + w])
        xy = np.concatenate(blocks, axis=1)
        in_maps.append({"xy": np.ascontiguousarray(xy)})
    return in_maps


def _combine(results):
    total = 0.0
    for c in range(_NCORES):
        out = np.asarray(results[c]["out"], dtype=np.float64)  # [8, 2]
        total += out[:, 0].sum()
        total += _K * out[:, 1].sum()
        if _ACT_FIN:
            total += np.asarray(results[c]["out2"], dtype=np.float64).sum()
    return np.float32(total / (_B * _K))


def kernel(pred_logits, gts):
    from concourse.bass_utils import run_bass_kernel_spmd

    global _cached_nc
    if _cached_nc is None:
        _cached_nc = build_bass()
    in_maps = _shard_inputs(pred_logits, gts)
    res = run_bass_kernel_spmd(_cached_nc, in_maps, list(range(_NCORES)))
    return _combine(res.results)


# revision 13
# speedup vs baseline: 2.8332x; 1.0002x over previous
"""Bootstrapped BCE-with-logits loss (top-25% hard-pixel mining) on 8 TRN2 cores.

Math: loss_pixel = softplus(x) - x*y  (== max(x,0) - x*y + log1p(exp(-|x|)))
For each row, mean of top-k (k = N/4) pixel losses, then global mean.

Key identity: with t = k-th largest value of row v,
    sum_topk(row) = k*t + sum_j relu(v_j - t)
and the RHS is stationary in t at t* (d/dt = k - count(v > t) = 0), so an
approximate per-row threshold gives only O(rho*N*delta^2) error.  The kernel
estimates t per row with a Newton iteration on subsample counts (on-device),
then does one fused relu pass.

Sharding: data-parallel over the batch dim: core c handles rows 8c..8c+7.
Each core's 8 rows are laid out as SBUF [128 partitions x 16384], partition
p holding elements of row p//16.  Inputs are cast to bf16 on the host
(halves DMA traffic; validated rel-err ~5e-5 vs f32 reference).

Engine assignment (v2):
  ACT:   exp + ln(e+1)  (softplus; no native softplus LUT in this PWP build)
  DVE:   p = x*y (tt 2x), V = sp - p (tt 2x), rl = max(V - t, 0) (ts 4x),
         Newton count passes (ts 4x + accum)
  PE:    per-row accumulation of rl: psum[8, 512] += ind8.T @ rl_slice
         (replaces a 2nd DVE accum pass + shrinks the serial tail)
Per-core output is [8, 2] f32: col0 = per-row sum of relu(V - t),
col1 = per-row threshold t.  Host: total = sum(col0) + K*sum(col1);
answer = total / (B*K).
"""

import numpy as np
import ml_dtypes

_NCORES = 8
_B = 64
_HW = 512 * 512            # 262144 pixels per row
_RPC = _B // _NCORES       # 8 rows per core
_P = 128                   # SBUF partitions
_FREE = _RPC * _HW // _P   # 16384 elements per partition
# tapered chunk widths: small first chunk lets ACT start early, equal middle
# chunks keep DMA ahead of ACT, small last chunk shrinks the serial tail
_CHUNK_W = [2048, 4096, 4096, 4096, 2048]
_CHUNK_OFF = [sum(_CHUNK_W[:i]) for i in range(len(_CHUNK_W))]
_NCHUNK = len(_CHUNK_W)
assert sum(_CHUNK_W) == _FREE
_K = _HW // 4              # 65536 (top-k per row)
_PPR = _P // _RPC          # 16 partitions per row

# chunk indices whose final (relu+accumulate) runs on the ACT engine
# (per-partition accum, summed on host) instead of DVE+PE: the trailing
# chunks, so the serial tail after the last Ln is one short ACT pass.
_ACT_FIN = frozenset()

# Newton refinement: (chunk index used for counting, sample width, half-width
# h, clamp on the update).  t0 is a distribution-informed initial guess.
_T0 = 0.92
_ROUNDS = [(1, 2048, 0.12, 0.50)]

_BF16 = ml_dtypes.bfloat16

# input-chunk double-buffer depth
_IO_BUFS = 3
# softplus-output buffer depth (ACT produces, DVE lags)
_SP_BUFS = 4
_ACC_COLS = 512            # PSUM accumulator lanes per row

_cached_nc = None


def build_bass(reps=1, loop=None):
    """Build the (SPMD, per-core identical) Bass program.

    reps > 1 repeats the whole body serially inside one NEFF; loop = N
    additionally wraps the body in a tc.For_i hardware loop (constant NEFF
    size at any rep count) — both only used for device-time measurement.
    """
    from concourse import bacc, mybir
    from concourse.tile import TileContext

    dt = mybir.dt
    Act = mybir.ActivationFunctionType
    Alu = mybir.AluOpType

    nc = bacc.Bacc("TRN2", target_bir_lowering=False, debug=False)

    # x and y interleaved per chunk ([..., j, 0, :] = x-chunk j, [..., 1, :] = y)
    # so each chunk needs exactly ONE dma_start.
    xy_ext = nc.declare_dram_parameter(
        "xy", [_P, 2 * _FREE], dt.bfloat16, isOutput=False
    )
    out_ext = nc.declare_dram_parameter("out", [_RPC, 2], dt.float32, isOutput=True)
    # per-partition relu sums from the ACT-final chunks (host sums them)
    n_act_fin = len(_ACT_FIN)
    out2_ext = nc.declare_dram_parameter(
        "out2", [_P, max(n_act_fin, 1)], dt.float32, isOutput=True
    )

    with TileContext(nc) as tc:
        with (
            tc.tile_pool(name="io", bufs=_IO_BUFS) as io_pool,
            tc.tile_pool(name="tmp", bufs=2) as tmp_pool,
            tc.tile_pool(name="spp", bufs=_SP_BUFS) as sp_pool,
            tc.tile_pool(name="persist", bufs=1) as persist,
            tc.tile_pool(name="small", bufs=1) as small,
            tc.tile_pool(name="psum", bufs=2, space="PSUM") as psum_pool,
            tc.tile_pool(name="psacc", bufs=1, space="PSUM") as psacc_pool,
        ):
            # persistent loss tile: all 8 rows of this core
            V = persist.tile([_P, _FREE], dt.bfloat16)

            # constants: row-indicator matrices for cross-partition
            # (per-row) reductions/broadcasts via the tensor engine.
            # ind8[p, b] = (p//16 == b), ind8T[b, p] = (p//16 == b)
            ind8 = small.tile([_P, _RPC], dt.float32)     # [128, 8] (Newton)
            ind8b = small.tile([_P, _RPC], dt.bfloat16)   # [128, 8] (finals)
            ind8T = small.tile([_RPC, _P], dt.float32)    # [8, 128]
            rid = small.tile([_P, 1], dt.int32)
            nc.gpsimd.iota(rid[:], [[0, 1]], channel_multiplier=1)
            nc.vector.tensor_scalar(
                rid[:], rid[:], 4, None, Alu.logical_shift_right
            )
            rid_f = small.tile([_P, 1], dt.float32)
            nc.vector.tensor_copy(rid_f[:], rid[:])
            col8 = small.tile([_P, _RPC], dt.int32)
            nc.gpsimd.iota(col8[:], [[1, _RPC]], channel_multiplier=0)
            col8_f = small.tile([_P, _RPC], dt.float32)
            nc.vector.tensor_copy(col8_f[:], col8[:])
            nc.vector.tensor_scalar(
                ind8[:], col8_f[:], rid_f[:], None, Alu.is_equal
            )
            nc.vector.tensor_copy(ind8b[:], ind8[:])
            colP = small.tile([_RPC, _P], dt.int32)
            nc.gpsimd.iota(colP[:], [[1, _P]], channel_multiplier=0)
            nc.vector.tensor_scalar(
                colP[:], colP[:], 4, None, Alu.logical_shift_right
            )
            rid8 = small.tile([_RPC, 1], dt.int32)
            nc.gpsimd.iota(rid8[:], [[0, 1]], channel_multiplier=1)
            rid8_f = small.tile([_RPC, 1], dt.float32)
            nc.vector.tensor_copy(rid8_f[:], rid8[:])
            colP_f = small.tile([_RPC, _P], dt.float32)
            nc.vector.tensor_copy(colP_f[:], colP[:])
            nc.vector.tensor_scalar(
                ind8T[:], colP_f[:], rid8_f[:], None, Alu.is_equal
            )

            # current per-row threshold, broadcast across the row's partitions
            t_bc = small.tile([_P, 1], dt.float32)
            t8 = small.tile([_RPC, 1], dt.float32)
            neg_t = small.tile([_P, 1], dt.float32)

            def produce_chunk(j):
                w, off = _CHUNK_W[j], _CHUNK_OFF[j]
                xyt = io_pool.tile([_P, 2 * w], dt.bfloat16, tag="xyt")
                nc.sync.dma_start(
                    xyt[:], xy_ext[:, 2 * off:2 * off + 2 * w]
                )
                xt = xyt[:, 0:w]
                yt = xyt[:, w:2 * w]
                sp = sp_pool.tile([_P, w], dt.bfloat16, tag="sp")
                e = tmp_pool.tile([_P, w], dt.float32, tag="e")
                nc.scalar.activation(e[:], xt, Act.Exp)
                nc.scalar.activation(sp[:], e[:], Act.Ln, bias=1.0)
                p = tmp_pool.tile([_P, w], dt.bfloat16, tag="p")
                nc.vector.tensor_tensor(p[:], xt, yt, Alu.mult)
                nc.vector.tensor_tensor(
                    V[:, off:off + w], sp[:], p[:], Alu.subtract
                )

            def newton_round(ridx, chunk, width, h, clamp):
                vc = V[:, _CHUNK_OFF[chunk]:_CHUNK_OFF[chunk] + width]
                n_samp = width * _PPR  # per-row sample count (over 16 parts)
                cnt = small.tile([_P, 3], dt.float32, tag=f"cnt{ridx}")
                msk = tmp_pool.tile([_P, width], dt.bfloat16, tag=f"msk{ridx}")
                # count = sum is_ge(v, thr); 3 thresholds around t0
                assert ridx == 0
                for i, off in enumerate((-h, 0.0, h)):
                    nc.vector.tensor_scalar(
                        msk[:], vc, float(_T0 + off), None, Alu.is_ge,
                        Alu.add, accum_out=cnt[:, i:i + 1],
                    )
                # per-row counts: [8, 3] = ind8.T @ cnt
                pc = psum_pool.tile([_RPC, 3], dt.float32, tag="pc")
                nc.tensor.matmul(pc[:], ind8[:], cnt[:])
                rc = small.tile([_RPC, 3], dt.float32, tag=f"rc{ridx}")
                nc.vector.tensor_copy(rc[:], pc[:])
                # Newton update: t += clamp(2h*(c_mid - n/4)/(c_lo - c_hi))
                num = small.tile([_RPC, 1], dt.float32, tag=f"num{ridx}")
                den = small.tile([_RPC, 1], dt.float32, tag=f"den{ridx}")
                q = small.tile([_RPC, 1], dt.float32, tag=f"q{ridx}")
                nc.vector.tensor_scalar(
                    num[:], rc[:, 1:2], float(n_samp / 4), float(2.0 * h),
                    Alu.subtract, Alu.mult,
                )
                nc.vector.tensor_tensor(den[:], rc[:, 0:1], rc[:, 2:3], Alu.subtract)
                rden = small.tile([_RPC, 1], dt.float32, tag=f"rden{ridx}")
                nc.vector.reciprocal(rden[:], den[:])
                nc.vector.tensor_tensor(q[:], num[:], rden[:], Alu.mult)
                nc.vector.tensor_scalar(
                    q[:], q[:], float(clamp), float(-clamp), Alu.min, Alu.max
                )
                nc.vector.tensor_tensor(t8[:], t8[:], q[:], Alu.add)
                # broadcast t8 [8,1] -> t_bc [128,1]
                pt = psum_pool.tile([_P, 1], dt.float32, tag="pt")
                nc.tensor.matmul(pt[:], ind8T[:], t8[:])
                nc.vector.tensor_copy(t_bc[:], pt[:])

            # final pass: rl = max(V - t, 0) on DVE (ts 4x), then per-row
            # accumulation on the tensor engine: ps[8, 512] += ind8b.T @ rl.
            # Chunks in _ACT_FIN instead run Relu(bias=-t)+accum on ACT
            # (per-partition sums, combined on host) — shrinks the tail.
            n_mm_total = sum(
                (w + _ACC_COLS - 1) // _ACC_COLS
                for j, w in enumerate(_CHUNK_W) if j not in _ACT_FIN
            )

            def make_final(ps, acc, mm_state):
                def final_chunk(j):
                    w, off = _CHUNK_W[j], _CHUNK_OFF[j]
                    if j in _ACT_FIN:
                        ci = sorted(_ACT_FIN).index(j)
                        rl = tmp_pool.tile([_P, w], dt.bfloat16, tag="rl")
                        nc.scalar.activation(
                            rl[:], V[:, off:off + w], Act.Relu,
                            bias=neg_t[:], accum_out=acc[:, ci:ci + 1],
                        )
                        return
                    rl = tmp_pool.tile([_P, w], dt.bfloat16, tag="rl")
                    nc.vector.tensor_scalar(
                        rl[:], V[:, off:off + w], t_bc[:], 0.0,
                        Alu.subtract, Alu.max,
                    )
                    for s in range(0, w, _ACC_COLS):
                        sw = min(_ACC_COLS, w - s)
                        nc.tensor.matmul(
                            ps[:, 0:sw], ind8b[:], rl[:, s:s + sw],
                            start=(mm_state[0] == 0),
                            stop=(mm_state[0] == n_mm_total - 1),
                        )
                        mm_state[0] += 1
                return final_chunk

            round_chunks = max(r[0] for r in _ROUNDS) + 1 if _ROUNDS else 0

            def rep_body():
                nc.vector.memset(t_bc[:], _T0)
                nc.vector.memset(t8[:], _T0)
                ps = psacc_pool.tile([_RPC, _ACC_COLS], dt.float32, tag="ps")
                acc = small.tile([_P, max(n_act_fin, 1)], dt.float32, tag="acc")
                if n_act_fin == 0:
                    nc.vector.memset(acc[:], 0.0)
                mm_state = [0]
                final_chunk = make_final(ps, acc, mm_state)
                for j in range(round_chunks):
                    produce_chunk(j)
                for ridx, (chunk, width, h, clamp) in enumerate(_ROUNDS):
                    newton_round(ridx, chunk, width, h, clamp)
                nc.vector.tensor_scalar_mul(neg_t[:], t_bc[:], -1.0)
                for j in range(round_chunks):
                    final_chunk(j)
                for j in range(round_chunks, _NCHUNK):
                    produce_chunk(j)
                    final_chunk(j)
                assert mm_state[0] == n_mm_total

                out_t = small.tile([_RPC, 2], dt.float32, tag="out_t")
                nc.vector.tensor_reduce(
                    out_t[:, 0:1], ps[:], mybir.AxisListType.X, Alu.add
                )
                nc.vector.tensor_copy(out_t[:, 1:2], t8[:])
                nc.sync.dma_start(out_ext[:], out_t[:])
                nc.sync.dma_start(out2_ext[:], acc[:])

            if loop is None:
                for _rep in range(reps):
                    rep_body()
            else:
                with tc.For_i(0, loop, 1):
                    for _rep in range(reps):
                        rep_body()

    # Steer the ACT table chooser to the one set holding BOTH Exp and Ln so
    # it loads a single table instead of thrashing exp<->ln sets per chunk.
    from concourse import bacc as _bacc_mod
    _orig_tables = _bacc_mod.get_activation_tables

    def _steered_tables(arch):
        tabs = dict(_orig_tables(arch))
        used = {Act.Exp, Act.Ln, Act.Sign, Act.Relu}
        combined = [n for n, fns in tabs.items() if {Act.Exp, Act.Ln} <= fns]
        if combined:
            keep = combined[0]
            tabs = {
                n: (fns if n == keep else (fns - used))
                for n, fns in tabs.items()
            }
        return tabs

    _bacc_mod.get_activation_tables = _steered_tables
    try:
        nc.compile()
    finally:
        _bacc_mod.get_activation_tables = _orig_tables
    return nc


def _shard_inputs(pred_logits, gts):
    x = np.ascontiguousarray(pred_logits, dtype=np.float32).reshape(_B, _HW)
    y = np.ascontiguousarray(gts, dtype=np.float32).reshape(_B, _HW)
    xb = x.astype(_BF16)
    yb = y.astype(_BF16)
    in_maps = []
    for c in range(_NCORES):
        sl = slice(c * _RPC, (c + 1) * _RPC)
        xs = xb[sl].reshape(_P, _FREE)
        ys = yb[sl].reshape(_P, _FREE)
        # interleave x/y per (variable-width) chunk: [... x_w | y_w ...]
        blocks = []
        for w, off in zip(_CHUNK_W, _CHUNK_OFF):
            blocks.append(xs[:, off:off + w])
            blocks.append(ys[:, off:off + w])
        xy = np.concatenate(blocks, axis=1)
        in_maps.append({"xy": np.ascontiguousarray(xy)})
    return in_maps


def _combine(results):
    total = 0.0
    for c in range(_NCORES):
        out = np.asarray(results[c]["out"], dtype=np.float64)  # [8, 2]
        total += out[:, 0].sum()
        total += _K * out[:, 1].sum()
        if _ACT_FIN:
            total += np.asarray(results[c]["out2"], dtype=np.float64).sum()
    return np.float32(total / (_B * _K))


def kernel(pred_logits, gts):
    from concourse.bass_utils import run_bass_kernel_spmd

    global _cached_nc
    if _cached_nc is None:
        _cached_nc = build_bass()
    in_maps = _shard_inputs(pred_logits, gts)
    res = run_bass_kernel_spmd(_cached_nc, in_maps, list(range(_NCORES)))
    return _combine(res.results)
